# revision 15
# baseline (speedup 1.0000x reference)
"""Trainium2 Bass kernel for nn_EulerCausalAttention_75892072121064.

Sharding: batch*heads across 8 cores (core c -> batch c//4, heads 4*(c%4)..+4).
Each core computes transposed-layout causal attention for its (b, 4-head)
slice plus the out-proj partial and writes outT (D, S); host sums the 4
per-batch partials and transposes back.

v2 design vs the earlier baseline:
- A custom DVE op (EULER_WRAP_ANT) fuses the whole angle-quantize/wrap chain
  into ONE instruction using f32 round-to-nearest-even tricks in "turn"
  units: t = x*C0 + C1; k = (t+3072)-3072 rounds to 1/4096ths (ulp trick);
  j = (k+3072^2)-3072^2 rounds to integers; w = k-j in [-1/2, 1/2] turns.
  Host-side row duplication (xdup) + per-partition constants make one
  [128, 1024] op + one ACT Sin(scale=2pi) emit a head's assembled [cos; sin]
  feature block directly (cos half pre-biased by +0.25 turns).
- bf16 for x/v_w (V path) and Q/K features: halves DMA and SBUF, full-rate
  matmuls; validated end-to-end error ~2.5e-3 (gate 2e-2).
- Exp on [128,1024] PSUM pairs (two 128-k blocks per ACT op) to amortize the
  ~290ns ACT per-op overhead. All Sins are emitted before all Exps so the
  ACT spline table loads exactly twice.
- Softmax denominator via a ones-column in the V tiles (row 64/63 of the
  attnV accumulation). Normalization never touches ACT: denominator rows are
  DMA-gathered into a [4,512] tile, batch-reciprocal'd on DVE
  (reciprocal_approx_accurate), partition-broadcast on Pool, and multiplied
  in-place on DVE. Odd heads accumulate attnV at partition offset 63 so all
  elementwise APs stay partition-aligned.
- out-proj contracts head pairs (K=128) from the normalized [128,512] f32r
  tiles, copied PSUM->SBUF on DVE and DMA'd to DRAM.
"""
import sys

import numpy as np

sys.path.insert(0, "/opt/trn_rl_repo")

import ml_dtypes  # noqa: E402

from concourse import bacc, mybir  # noqa: E402
import concourse.tile as tile  # noqa: E402
from concourse.bass_utils import run_bass_kernel_spmd  # noqa: E402
from concourse import dve_ops  # noqa: E402
from concourse.dve_spec import (  # noqa: E402
    C0, C1, C2, C3, Spec, Src0, _spill_c3_to_src1, lower,
)
from concourse.dve_uop import DveOpSpec  # noqa: E402

B, S, D, H, DH = 2, 2048, 1024, 16, 64
NCORES, HPC = 8, 4
CW = HPC * DH          # 256 feature cols per core
SQW = 512              # q window
NQW = S // SQW
NS = S // 128          # 128-row seq tiles
LUT = 4096
TWO_PI_F = float(np.float32(2.0 * np.pi))
SCALE = float(np.sqrt(np.float32(2.0 * DH)))
INV_SCALE = float(np.float32(1.0) / np.float32(SCALE))
RND_A = 3072.0                     # 1/4096-granularity rounder (ulp trick)
RND_B = float(np.float32(RND_A) * np.float32(RND_A))  # integer rounder

F32 = mybir.dt.float32
F32R = mybir.dt.float32r
BF16 = mybir.dt.bfloat16
I32 = mybir.dt.int32
AF = mybir.ActivationFunctionType
ALU = mybir.AluOpType
BF16NP = ml_dtypes.bfloat16

# Schraudolph exp offload: every Nth score-pair runs on DVE instead of ACT
# (0 = disabled). exp(z) ~ bitcast_f32(i32(z*A + B)).
SCHRAUD_EVERY = 0
SCH_A = float(np.float32(INV_SCALE * (2.0 ** 23) / np.log(2.0)))
SCH_B = float(np.float32(127.0 * 2.0 ** 23 - 366392.0))

_CACHE = {}


def _register_euler_wrap():
    for op in dve_ops.OPS:
        if op.name == "EULER_WRAP_ANT":
            return op
    t = Src0 * C0 + C1
    k = (t + C2) - C2
    j = (k + C3) - C3
    body = _spill_c3_to_src1(k - j)

    def ref(in0, in1, s0, s1, imm2):
        f32 = np.float32
        t = f32(f32(in0 * s0) + s1)
        k = f32(f32(t + imm2) - imm2)
        j = f32(f32(k + in1) - in1)
        return f32(k - j)

    spec = Spec(body=body, reference=ref)
    ups = lower(spec, ver="v3")
    row = max(dve_ops._SUB_OPCODE_FOR_NAME.values()) + 1
    sha = DveOpSpec(name="EULER_WRAP_ANT", opcode=row, uops=ups,
                    rd1_en=True).sha("v3")
    op = dve_ops.DveOp("EULER_WRAP_ANT", spec, subdim=False,
                       uops_sha={"v3": sha})
    dve_ops._SUB_OPCODE_FOR_NAME["EULER_WRAP_ANT"] = row
    dve_ops.OPS.append(op)
    dve_ops.CUSTOM_DVE_SPECS["EULER_WRAP_ANT"] = spec
    return op


EULER_WRAP = _register_euler_wrap()


def _build_nc():
    nc = bacc.Bacc("TRN2", debug=False, num_devices=NCORES)

    xf = nc.dram_tensor("xf", [CW, S], F32, kind="ExternalInput")
    xbr = nc.dram_tensor("xbr", [D - CW, S], BF16, kind="ExternalInput")
    vw = nc.dram_tensor("vw", [128, 8 * CW], BF16, kind="ExternalInput")
    ow = nc.dram_tensor("ow", [CW, D], F32R, kind="ExternalInput")
    cst = nc.dram_tensor("cst", [128, 16], F32, kind="ExternalInput")
    tri = nc.dram_tensor("tri", [128, 128], F32, kind="ExternalInput")
    oh = nc.dram_tensor("oh", [97, 256], F32R, kind="ExternalInput")
    outT = nc.dram_tensor("outT", [D, S], F32, kind="ExternalOutput")

    with tile.TileContext(nc) as tc:
        with (
            tc.tile_pool(name="pp", bufs=1) as pp,
            tc.tile_pool(name="att", bufs=1) as ap,
            tc.tile_pool(name="ps", bufs=1, space="PSUM") as ps,
        ):
            tri_sb = pp.tile([128, 128], F32, tag="tri")
            nc.sync.dma_start(tri_sb[:], tri[:])
            cst_sb = pp.tile([128, 16], F32, tag="cst")
            nc.sync.dma_start(cst_sb[:], cst[:])
            ccr = pp.tile([128, 1], F32, tag="ccr")
            nc.gpsimd.memset(ccr[:], RND_B)
            oh_sb = pp.tile([97, 256], F32R, tag="oh")
            nc.sync.dma_start(oh_sb[:], oh[:])
            vw_sb = pp.tile([128, 8 * CW], BF16, tag="vw")
            ow_sb = [pp.tile([128, D], F32R, tag=f"ow{hp}", name=f"ow{hp}")
                     for hp in range(2)]
            qt = [pp.tile([128, S], BF16, tag=f"qt{h}", name=f"qt{h}")
                  for h in range(HPC)]
            kt = [pp.tile([128, S], BF16, tag=f"kt{h}", name=f"kt{h}")
                  for h in range(HPC)]
            vt = [pp.tile([128, HPC * 65], F32R, tag=f"vt{si}", name=f"vt{si}")
                  for si in range(NS)]
            # per-head V column layout: [feats64, one] at 65*h
            VCOL = {h: 65 * h for h in range(HPC)}

            with tc.tile_pool(name="xp", bufs=1) as xp:
                xf_sb = [xp.tile([128, S], F32, tag=f"xf{od}", name=f"xf{od}")
                         for od in range(2)]
                xdup_sb = [xp.tile([128, S], F32, tag=f"xd{h}", name=f"xd{h}")
                           for h in range(HPC)]
                xtg = [xp.tile([128, S], BF16, tag=f"xt{od}", name=f"xt{od}")
                       for od in range(8)]

                # head-dim rows once in f32 (features need full precision)
                for od in range(2):
                    for half in range(2):
                        cs = slice(1024 * half, 1024 * half + 1024)
                        nc.sync.dma_start(xf_sb[od][:, cs],
                                          xf[od * 128:(od + 1) * 128, cs])
                # duplicate rows on-chip: xdup[h] = [dims(h); dims(h)]
                for h in range(HPC):
                    src = xf_sb[h // 2][64 * (h % 2):64 * (h % 2) + 64, :]
                    nc.sync.dma_start(xdup_sb[h][0:64, :], src)
                    nc.sync.dma_start(xdup_sb[h][64:128, :], src)
                nc.sync.dma_start(vw_sb[:], vw[:])
                # bf16 copies of the head-dim rows for the V matmul
                for od in range(2):
                    nc.gpsimd.tensor_copy(xtg[od][:], xf_sb[od][:])
                for od in range(2, 8):
                    for half in range(2):
                        cs = slice(1024 * half, 1024 * half + 1024)
                        nc.sync.dma_start(
                            xtg[od][:, cs],
                            xbr[(od - 2) * 128:(od - 1) * 128, cs])
                for hp in range(2):
                    nc.sync.dma_start(ow_sb[hp][:],
                                      ow[hp * 128:(hp + 1) * 128, :])

                # ---- Euler features: 1 custom DVE + 1 Sin per (h, qk, half)
                for half in range(2):
                    for h in range(HPC):
                        for qk in range(2):
                            dtile = (qt, kt)[qk][h]
                            c = 4 * h + 2 * qk
                            cs = slice(1024 * half, 1024 * half + 1024)
                            wt = xp.tile([128, 1024], F32, tag="wt", bufs=2,
                                         name="wt")
                            nc.vector._custom_dve(
                                EULER_WRAP, out=wt[:], in0=xdup_sb[h][:, cs],
                                in1=ccr[:], s0=cst_sb[:, c:c + 1],
                                s1=cst_sb[:, c + 1:c + 2], imm2=RND_A,
                            )
                            nc.scalar.activation(dtile[:, cs], wt[:], AF.Sin,
                                                 scale=TWO_PI_F)

                # ---- V = x @ vwT (bf16 inputs, f32 psum, f32r vt + ones) ----
                vwv = vw_sb[:].rearrange("p (od c) -> p od c", c=CW)
                for si in range(NS):
                    vps = ps.tile([128, CW], F32, tag="pr", name="vps", bufs=2)
                    for od in range(8):
                        nc.tensor.matmul(
                            vps[:],
                            xtg[od][:, si * 128:(si + 1) * 128],
                            vwv[:, od, :],
                            start=(od == 0), stop=(od == 7),
                        )
                    for h in range(HPC):
                        fc = VCOL[h]
                        nc.vector.tensor_copy(
                            vt[si][:, fc:fc + 64],
                            vps[:, h * 64:(h + 1) * 64],
                        )
                    ones = vt[si][:].rearrange(
                        "p (g w) -> p g w", w=65)[:, :, 64:65]
                    nc.gpsimd.memset(ones.bitcast(F32), 1.0)

            # ---- attention + projection ----
            pair_idx = 0
            for qw in range(NQW):
                nkb = 4 * qw + 4
                npair = nkb // 2
                dsb = ap.tile([97, SQW], F32, tag="dsb", bufs=2,
                              name="dsb")
                nc.gpsimd.memset(dsb[:], 1.0)
                praw = [ap.tile([128, SQW], F32R, tag=f"praw{hp}", bufs=2,
                                name=f"praw{hp}") for hp in range(2)]
                for h in range(HPC):
                    qsl = qt[h][:, qw * SQW:(qw + 1) * SQW]
                    ats = []
                    for kp in range(npair):
                        sc = ps.tile([128, 1024], F32, tag="sc", bufs=2,
                                     name="sc")
                        for half in range(2):
                            kb = 2 * kp + half
                            nc.tensor.matmul(
                                sc[:, 512 * half:512 * half + 512],
                                kt[h][:, kb * 128:(kb + 1) * 128],
                                qsl,
                                start=True, stop=True,
                            )
                        use_sch = (SCHRAUD_EVERY
                                   and pair_idx % SCHRAUD_EVERY
                                   == SCHRAUD_EVERY - 1)
                        pair_idx += 1
                        if use_sch:
                            ati = ap.tile([128, 1024], I32, tag="at", bufs=6,
                                          name="ati")
                            nc.vector.tensor_scalar(ati[:], sc[:], SCH_A,
                                                    SCH_B, ALU.mult, ALU.add)
                            at = ati[:].bitcast(F32R)
                        else:
                            att = ap.tile([128, 1024], F32R, tag="at", bufs=6,
                                          name="att")
                            nc.scalar.activation(att[:], sc[:], AF.Exp,
                                                 scale=INV_SCALE)
                            at = att[:]
                        for half in range(2):
                            kb = 2 * kp + half
                            r = kb - 4 * qw
                            if r >= 0:
                                cb = 512 * half
                                blk = at[:, cb + 128 * r:cb + 128 * r + 128]
                                nc.gpsimd.tensor_tensor(blk, blk, tri_sb[:],
                                                        ALU.mult)
                                if r > 0:
                                    nc.gpsimd.memset(
                                        at[:, cb:cb + 128 * r].bitcast(F32),
                                        0.0)
                        ats.append(at)

                    hp, odd = h // 2, h % 2
                    o = ps.tile([65, SQW], F32, tag=("oo" if odd else "oe"),
                                bufs=1, name="oo" if odd else "oe")
                    dst, feats, den = o[:], o[0:64, :], o[64:65, :]
                    for kb in range(nkb):
                        nc.tensor.matmul(
                            dst,
                            vt[kb][:, VCOL[h]:VCOL[h] + 65],
                            ats[kb // 2][:, 512 * (kb % 2):512 * (kb % 2) + 512],
                            start=(kb == 0), stop=(kb == nkb - 1),
                        )
                    nc.vector.tensor_copy(dsb[32 * h:32 * h + 1, :], den)
                    nc.vector.tensor_copy(
                        praw[hp][64 * odd:64 * odd + 64, :], feats)

                rec = ap.tile([97, SQW], F32, tag="rec", bufs=2,
                              name="rec")
                rscr = ap.tile([97, SQW], F32, tag="rscr", bufs=2,
                               name="rscr")
                nc.vector.reciprocal_approx_accurate(rec[:], dsb[:], rscr[:])
                recr = ap.tile([97, SQW], F32R, tag="recr", bufs=2,
                               name="recr")
                nc.vector.tensor_copy(recr[:], rec[:])
                for hp in range(2):
                    # bc rows 0:64 = 1/den[2hp], 64:128 = 1/den[2hp+1]
                    bc = ps.tile([128, SQW], F32, tag="oe", bufs=1, name="bc")
                    nc.tensor.matmul(bc[:], oh_sb[:, hp * 128:(hp + 1) * 128],
                                     recr[:],
                                     start=True, stop=True)
                    for odd in range(2):
                        sl = slice(64 * odd, 64 * odd + 64)
                        nc.vector.tensor_tensor(praw[hp][sl, :],
                                                praw[hp][sl, :],
                                                bc[sl, :], ALU.mult)

                for od in range(8):
                    pr = ps.tile([128, SQW], F32, tag="pr", bufs=2, name="pr")
                    for hp in range(2):
                        nc.tensor.matmul(
                            pr[:],
                            ow_sb[hp][:, od * 128:(od + 1) * 128],
                            praw[hp][:],
                            start=(hp == 0), stop=(hp == 1),
                        )
                    prsb = ap.tile([128, SQW], F32, tag="prsb", bufs=3,
                                   name="prsb")
                    nc.vector.tensor_copy(prsb[:], pr[:])
                    nc.sync.dma_start(
                        outT[od * 128:(od + 1) * 128,
                             qw * SQW:(qw + 1) * SQW],
                        prsb[:],
                    )

    nc.compile()
    return nc


def _prep_inputs(x, w_q, b_q, w_k, b_k, v_w, out_w):
    """Build the 8 per-core input maps (host-side sharding)."""
    in_maps = []
    tri = np.triu(np.ones((128, 128), dtype=np.float32))  # keep q>=k: g>=p
    inv2pi = np.float64(1.0) / (2.0 * np.pi)

    for c in range(NCORES):
        b = c // 4
        h0 = (c % 4) * HPC
        colbase = h0 * DH
        cols = np.arange(colbase, colbase + CW)
        rest = np.concatenate([np.arange(0, colbase),
                               np.arange(colbase + CW, D)])
        perm = np.concatenate([cols, rest])

        xbp = np.ascontiguousarray(x[b][:, perm].T)           # [D, S] f32
        xfv = np.ascontiguousarray(xbp[:CW])                  # f32 head dims
        xbrv = np.ascontiguousarray(xbp[CW:]).astype(BF16NP)  # bf16 rest

        vwp = v_w[cols][:, perm].T                            # [D, CW]
        vwt = np.ascontiguousarray(
            vwp.reshape(8, 128, CW).transpose(1, 0, 2).reshape(128, 8 * CW)
        ).astype(BF16NP)

        owp = np.ascontiguousarray(out_w[:, cols].T, dtype=np.float32)

        cstv = np.zeros((128, 16), dtype=np.float32)
        for h in range(HPC):
            for qk, (w_, b_) in enumerate(((w_q, b_q), (w_k, b_k))):
                wv = w_[h0 + h].astype(np.float64)
                bv = b_[h0 + h].astype(np.float64)
                c0 = (inv2pi / (1.0 + np.abs(wv))).astype(np.float32)
                c1s = (bv * inv2pi).astype(np.float32)
                c1c = (bv * inv2pi + 0.25).astype(np.float32)
                col = 4 * h + 2 * qk
                cstv[0:64, col] = c0
                cstv[64:128, col] = c0
                cstv[0:64, col + 1] = c1c
                cstv[64:128, col + 1] = c1s

        ohv = np.zeros((97, 256), dtype=np.float32)
        for hp in range(2):
            ohv[32 * (2 * hp), hp * 128:hp * 128 + 64] = 1.0
            ohv[32 * (2 * hp + 1), hp * 128 + 64:hp * 128 + 128] = 1.0

        in_maps.append(dict(
            xf=xfv, xbr=xbrv, vw=vwt, ow=owp, cst=cstv, tri=tri, oh=ohv,
        ))
    return in_maps


def kernel(x, w_q, b_q, w_k, b_k, v_w, out_w, _trace=False):
    x = np.asarray(x, dtype=np.float32)
    w_q = np.asarray(w_q, dtype=np.float32)
    b_q = np.asarray(b_q, dtype=np.float32)
    w_k = np.asarray(w_k, dtype=np.float32)
    b_k = np.asarray(b_k, dtype=np.float32)
    v_w = np.asarray(v_w, dtype=np.float32)
    out_w = np.asarray(out_w, dtype=np.float32)

    if "nc" not in _CACHE:
        _CACHE["nc"] = _build_nc()
    nc = _CACHE["nc"]

    in_maps = _prep_inputs(x, w_q, b_q, w_k, b_k, v_w, out_w)
    res = run_bass_kernel_spmd(
        nc, in_maps, core_ids=list(range(NCORES)), trace=_trace
    )
    out = np.zeros((B, S, D), dtype=np.float32)
    for c in range(NCORES):
        out[c // 4] += res.results[c]["outT"].T
    if _trace:
        kernel._last_result = res
    return out


# revision 16
# speedup vs baseline: 1.1718x; 1.1718x over previous
"""Trainium2 Bass kernel for nn_EulerCausalAttention_75892072121064.

Sharding: batch*heads across 8 cores (core c -> batch c//4, heads 4*(c%4)..+4).
Each core computes transposed-layout causal attention for its (b, 4-head)
slice plus the out-proj partial and writes outT (D, S); host sums the 4
per-batch partials and transposes back.

v2 design vs the earlier baseline:
- A custom DVE op (EULER_WRAP_ANT) fuses the whole angle-quantize/wrap chain
  into ONE instruction using f32 round-to-nearest-even tricks in "turn"
  units: t = x*C0 + C1; k = (t+3072)-3072 rounds to 1/4096ths (ulp trick);
  j = (k+3072^2)-3072^2 rounds to integers; w = k-j in [-1/2, 1/2] turns.
  Host-side row duplication (xdup) + per-partition constants make one
  [128, 1024] op + one ACT Sin(scale=2pi) emit a head's assembled [cos; sin]
  feature block directly (cos half pre-biased by +0.25 turns).
- bf16 for x/v_w (V path) and Q/K features: halves DMA and SBUF, full-rate
  matmuls; validated end-to-end error ~2.5e-3 (gate 2e-2).
- Exp on [128,1024] PSUM pairs (two 128-k blocks per ACT op) to amortize the
  ~290ns ACT per-op overhead. All Sins are emitted before all Exps so the
  ACT spline table loads exactly twice.
- Softmax denominator via a ones-column in the V tiles (row 64/63 of the
  attnV accumulation). Normalization never touches ACT: denominator rows are
  DMA-gathered into a [4,512] tile, batch-reciprocal'd on DVE
  (reciprocal_approx_accurate), partition-broadcast on Pool, and multiplied
  in-place on DVE. Odd heads accumulate attnV at partition offset 63 so all
  elementwise APs stay partition-aligned.
- out-proj contracts head pairs (K=128) from the normalized [128,512] f32r
  tiles, copied PSUM->SBUF on DVE and DMA'd to DRAM.
"""
import sys

import numpy as np

sys.path.insert(0, "/opt/trn_rl_repo")

import ml_dtypes  # noqa: E402

from concourse import bacc, mybir  # noqa: E402
import concourse.tile as tile  # noqa: E402
from concourse.bass_utils import run_bass_kernel_spmd  # noqa: E402
from concourse import dve_ops  # noqa: E402
from concourse.dve_spec import (  # noqa: E402
    C0, C1, C2, C3, Spec, Src0, _spill_c3_to_src1, lower,
)
from concourse.dve_uop import DveOpSpec  # noqa: E402

B, S, D, H, DH = 2, 2048, 1024, 16, 64
NCORES, HPC = 8, 4
CW = HPC * DH          # 256 feature cols per core
SQW = 512              # q window
NQW = S // SQW
NS = S // 128          # 128-row seq tiles
LUT = 4096
TWO_PI_F = float(np.float32(2.0 * np.pi))
SCALE = float(np.sqrt(np.float32(2.0 * DH)))
INV_SCALE = float(np.float32(1.0) / np.float32(SCALE))
RND_A = 3072.0                     # 1/4096-granularity rounder (ulp trick)
RND_B = float(np.float32(RND_A) * np.float32(RND_A))  # integer rounder

F32 = mybir.dt.float32
F32R = mybir.dt.float32r
BF16 = mybir.dt.bfloat16
I32 = mybir.dt.int32
AF = mybir.ActivationFunctionType
ALU = mybir.AluOpType
BF16NP = ml_dtypes.bfloat16

# Schraudolph exp offload: every Nth score-pair runs on DVE instead of ACT
# (0 = disabled). exp(z) ~ bitcast_f32(i32(z*A + B)).
SCHRAUD_EVERY = 0
SCH_A = float(np.float32(INV_SCALE * (2.0 ** 23) / np.log(2.0)))
SCH_B = float(np.float32(127.0 * 2.0 ** 23 - 366392.0))

_CACHE = {}


def _register_euler_wrap():
    for op in dve_ops.OPS:
        if op.name == "EULER_WRAP_ANT":
            return op
    t = Src0 * C0 + C1
    k = (t + C2) - C2
    j = (k + C3) - C3
    body = _spill_c3_to_src1(k - j)

    def ref(in0, in1, s0, s1, imm2):
        f32 = np.float32
        t = f32(f32(in0 * s0) + s1)
        k = f32(f32(t + imm2) - imm2)
        j = f32(f32(k + in1) - in1)
        return f32(k - j)

    spec = Spec(body=body, reference=ref)
    ups = lower(spec, ver="v3")
    row = max(dve_ops._SUB_OPCODE_FOR_NAME.values()) + 1
    sha = DveOpSpec(name="EULER_WRAP_ANT", opcode=row, uops=ups,
                    rd1_en=True).sha("v3")
    op = dve_ops.DveOp("EULER_WRAP_ANT", spec, subdim=False,
                       uops_sha={"v3": sha})
    dve_ops._SUB_OPCODE_FOR_NAME["EULER_WRAP_ANT"] = row
    dve_ops.OPS.append(op)
    dve_ops.CUSTOM_DVE_SPECS["EULER_WRAP_ANT"] = spec
    return op


EULER_WRAP = _register_euler_wrap()


def _build_nc():
    nc = bacc.Bacc("TRN2", debug=False, num_devices=NCORES)

    xb = nc.dram_tensor("xb", [D, S], BF16, kind="ExternalInput")
    xdup = nc.dram_tensor("xdup", [128, HPC * S], F32, kind="ExternalInput")
    vw = nc.dram_tensor("vw", [128, 8 * CW], BF16, kind="ExternalInput")
    ow = nc.dram_tensor("ow", [CW, D], F32R, kind="ExternalInput")
    cst = nc.dram_tensor("cst", [128, 16], F32, kind="ExternalInput")
    tri = nc.dram_tensor("tri", [128, 128], F32, kind="ExternalInput")
    oh = nc.dram_tensor("oh", [97, 256], F32R, kind="ExternalInput")
    outT = nc.dram_tensor("outT", [D, S], F32, kind="ExternalOutput")

    with tile.TileContext(nc) as tc:
        with (
            tc.tile_pool(name="pp", bufs=1) as pp,
            tc.tile_pool(name="att", bufs=1) as ap,
            tc.tile_pool(name="ps", bufs=1, space="PSUM") as ps,
        ):
            tri_sb = pp.tile([128, 128], F32, tag="tri")
            nc.sync.dma_start(tri_sb[:], tri[:])
            cst_sb = pp.tile([128, 16], F32, tag="cst")
            nc.sync.dma_start(cst_sb[:], cst[:])
            ccr = pp.tile([128, 1], F32, tag="ccr")
            nc.gpsimd.memset(ccr[:], RND_B)
            oh_sb = pp.tile([97, 256], F32R, tag="oh")
            nc.sync.dma_start(oh_sb[:], oh[:])
            vw_sb = pp.tile([128, 8 * CW], BF16, tag="vw")
            ow_sb = [pp.tile([128, D], F32R, tag=f"ow{hp}", name=f"ow{hp}")
                     for hp in range(2)]
            qt = [pp.tile([128, S], BF16, tag=f"qt{h}", name=f"qt{h}")
                  for h in range(HPC)]
            kt = [pp.tile([128, S], BF16, tag=f"kt{h}", name=f"kt{h}")
                  for h in range(HPC)]
            vt = [pp.tile([128, HPC * 65], F32R, tag=f"vt{si}", name=f"vt{si}")
                  for si in range(NS)]
            # per-head V column layout: [feats64, one] at 65*h
            VCOL = {h: 65 * h for h in range(HPC)}

            with tc.tile_pool(name="xp", bufs=1) as xp:
                # warmup: first custom-DVE op pays ~7.5us ucode load; run a
                # tiny one immediately (deps: ccr memset only).
                wrm = xp.tile([128, 1], F32, tag="wrm")
                nc.vector._custom_dve(
                    EULER_WRAP, out=wrm[:], in0=ccr[:], in1=ccr[:],
                    s0=0.001, s1=0.0, imm2=RND_A,
                )
                xdup_sb = [xp.tile([128, S], F32, tag=f"xd{h}", name=f"xd{h}")
                           for h in range(HPC)]
                xtg = [xp.tile([128, S], BF16, tag=f"xt{od}", name=f"xt{od}")
                       for od in range(8)]

                def dma_xdup(h, half):
                    cs = slice(1024 * half, 1024 * half + 1024)
                    nc.sync.dma_start(
                        xdup_sb[h][:, cs],
                        xdup[:, S * h + 1024 * half:
                             S * h + 1024 * half + 1024],
                    )

                def dma_xb(od, half):
                    cs = slice(1024 * half, 1024 * half + 1024)
                    nc.sync.dma_start(xtg[od][:, cs],
                                      xb[od * 128:(od + 1) * 128, cs])

                # order: feature inputs (half0) first so Sins start early,
                # then V inputs for cols 0:1024, then the rest.
                for h in range(HPC):
                    dma_xdup(h, 0)
                nc.sync.dma_start(vw_sb[:], vw[:])
                for od in range(8):
                    dma_xb(od, 0)
                for h in range(HPC):
                    dma_xdup(h, 1)
                for hp in range(2):
                    nc.sync.dma_start(ow_sb[hp][:],
                                      ow[hp * 128:(hp + 1) * 128, :])
                for od in range(8):
                    dma_xb(od, 1)

                # ---- Euler features: 1 custom DVE + 1 Sin per (h, qk, half)
                for half in range(2):
                    for h in range(HPC):
                        for qk in range(2):
                            dtile = (qt, kt)[qk][h]
                            c = 4 * h + 2 * qk
                            cs = slice(1024 * half, 1024 * half + 1024)
                            wt = xp.tile([128, 1024], F32, tag="wt", bufs=4,
                                         name="wt")
                            nc.vector._custom_dve(
                                EULER_WRAP, out=wt[:], in0=xdup_sb[h][:, cs],
                                in1=ccr[:], s0=cst_sb[:, c:c + 1],
                                s1=cst_sb[:, c + 1:c + 2], imm2=RND_A,
                            )
                            nc.scalar.activation(dtile[:, cs], wt[:], AF.Sin,
                                                 scale=TWO_PI_F)

                # ---- V = x @ vwT (bf16 inputs, f32 psum, f32r vt + ones) ----
                vwv = vw_sb[:].rearrange("p (od c) -> p od c", c=CW)
                for si in range(NS):
                    vps = ps.tile([128, CW], F32, tag="pr", name="vps", bufs=2)
                    for od in range(8):
                        nc.tensor.matmul(
                            vps[:],
                            xtg[od][:, si * 128:(si + 1) * 128],
                            vwv[:, od, :],
                            start=(od == 0), stop=(od == 7),
                        )
                    for h in range(HPC):
                        fc = VCOL[h]
                        nc.vector.tensor_copy(
                            vt[si][:, fc:fc + 64],
                            vps[:, h * 64:(h + 1) * 64],
                        )
                    ones = vt[si][:].rearrange(
                        "p (g w) -> p g w", w=65)[:, :, 64:65]
                    nc.gpsimd.memset(ones.bitcast(F32), 1.0)

            # ---- attention + projection ----
            pair_idx = 0
            for qw in range(NQW):
                nkb = 4 * qw + 4
                npair = nkb // 2
                dsb = ap.tile([97, SQW], F32, tag="dsb", bufs=2,
                              name="dsb")
                nc.gpsimd.memset(dsb[:], 1.0)
                praw = [ap.tile([128, SQW], F32R, tag=f"praw{hp}", bufs=2,
                                name=f"praw{hp}") for hp in range(2)]
                for h in range(HPC):
                    qsl = qt[h][:, qw * SQW:(qw + 1) * SQW]
                    ats = []
                    for kp in range(npair):
                        sc = ps.tile([128, 1024], F32, tag="sc", bufs=2,
                                     name="sc")
                        for half in range(2):
                            kb = 2 * kp + half
                            nc.tensor.matmul(
                                sc[:, 512 * half:512 * half + 512],
                                kt[h][:, kb * 128:(kb + 1) * 128],
                                qsl,
                                start=True, stop=True,
                            )
                        use_sch = (SCHRAUD_EVERY
                                   and pair_idx % SCHRAUD_EVERY
                                   == SCHRAUD_EVERY - 1)
                        pair_idx += 1
                        if use_sch:
                            ati = ap.tile([128, 1024], I32, tag="at", bufs=8,
                                          name="ati")
                            nc.vector.tensor_scalar(ati[:], sc[:], SCH_A,
                                                    SCH_B, ALU.mult, ALU.add)
                            at = ati[:].bitcast(F32R)
                        else:
                            att = ap.tile([128, 1024], F32R, tag="at", bufs=8,
                                          name="att")
                            nc.scalar.activation(att[:], sc[:], AF.Exp,
                                                 scale=INV_SCALE)
                            at = att[:]
                        for half in range(2):
                            kb = 2 * kp + half
                            r = kb - 4 * qw
                            if r >= 0:
                                cb = 512 * half
                                blk = at[:, cb + 128 * r:cb + 128 * r + 128]
                                nc.gpsimd.tensor_tensor(blk, blk, tri_sb[:],
                                                        ALU.mult)
                                if r > 0:
                                    nc.gpsimd.memset(
                                        at[:, cb:cb + 128 * r].bitcast(F32),
                                        0.0)
                        ats.append(at)

                    hp, odd = h // 2, h % 2
                    o = ps.tile([65, SQW], F32, tag=("oo" if odd else "oe"),
                                bufs=1, name="oo" if odd else "oe")
                    dst, feats, den = o[:], o[0:64, :], o[64:65, :]
                    for kb in range(nkb):
                        nc.tensor.matmul(
                            dst,
                            vt[kb][:, VCOL[h]:VCOL[h] + 65],
                            ats[kb // 2][:, 512 * (kb % 2):512 * (kb % 2) + 512],
                            start=(kb == 0), stop=(kb == nkb - 1),
                        )
                    nc.vector.tensor_copy(dsb[32 * h:32 * h + 1, :], den)
                    nc.vector.tensor_copy(
                        praw[hp][64 * odd:64 * odd + 64, :], feats)

                rec = ap.tile([97, SQW], F32, tag="rec", bufs=2,
                              name="rec")
                rscr = ap.tile([97, SQW], F32, tag="rscr", bufs=2,
                               name="rscr")
                nc.vector.reciprocal_approx_accurate(rec[:], dsb[:], rscr[:])
                recr = ap.tile([97, SQW], F32R, tag="recr", bufs=2,
                               name="recr")
                nc.vector.tensor_copy(recr[:], rec[:])
                for hp in range(2):
                    # bc rows 0:64 = 1/den[2hp], 64:128 = 1/den[2hp+1]
                    bc = ps.tile([128, SQW], F32, tag="oe", bufs=1, name="bc")
                    nc.tensor.matmul(bc[:], oh_sb[:, hp * 128:(hp + 1) * 128],
                                     recr[:],
                                     start=True, stop=True)
                    for odd in range(2):
                        sl = slice(64 * odd, 64 * odd + 64)
                        nc.vector.tensor_tensor(praw[hp][sl, :],
                                                praw[hp][sl, :],
                                                bc[sl, :], ALU.mult)

                for od in range(8):
                    pr = ps.tile([128, SQW], F32, tag="pr", bufs=2, name="pr")
                    for hp in range(2):
                        nc.tensor.matmul(
                            pr[:],
                            ow_sb[hp][:, od * 128:(od + 1) * 128],
                            praw[hp][:],
                            start=(hp == 0), stop=(hp == 1),
                        )
                    prsb = ap.tile([128, SQW], F32, tag="prsb", bufs=3,
                                   name="prsb")
                    nc.vector.tensor_copy(prsb[:], pr[:])
                    nc.sync.dma_start(
                        outT[od * 128:(od + 1) * 128,
                             qw * SQW:(qw + 1) * SQW],
                        prsb[:],
                    )

    nc.compile()
    return nc


def _prep_inputs(x, w_q, b_q, w_k, b_k, v_w, out_w):
    """Build the 8 per-core input maps (host-side sharding)."""
    in_maps = []
    tri = np.triu(np.ones((128, 128), dtype=np.float32))  # keep q>=k: g>=p
    inv2pi = np.float64(1.0) / (2.0 * np.pi)

    for c in range(NCORES):
        b = c // 4
        h0 = (c % 4) * HPC
        colbase = h0 * DH
        cols = np.arange(colbase, colbase + CW)
        rest = np.concatenate([np.arange(0, colbase),
                               np.arange(colbase + CW, D)])
        perm = np.concatenate([cols, rest])

        xbp = np.ascontiguousarray(x[b][:, perm].T)           # [D, S] f32
        xb_bf = xbp.astype(BF16NP)

        xdupv = np.empty((128, HPC * S), dtype=np.float32)
        for h in range(HPC):
            xh = xbp[h * DH:(h + 1) * DH, :]                  # [64, S]
            xdupv[0:64, h * S:(h + 1) * S] = xh
            xdupv[64:128, h * S:(h + 1) * S] = xh

        vwp = v_w[cols][:, perm].T                            # [D, CW]
        vwt = np.ascontiguousarray(
            vwp.reshape(8, 128, CW).transpose(1, 0, 2).reshape(128, 8 * CW)
        ).astype(BF16NP)

        owp = np.ascontiguousarray(out_w[:, cols].T, dtype=np.float32)

        cstv = np.zeros((128, 16), dtype=np.float32)
        for h in range(HPC):
            for qk, (w_, b_) in enumerate(((w_q, b_q), (w_k, b_k))):
                wv = w_[h0 + h].astype(np.float64)
                bv = b_[h0 + h].astype(np.float64)
                c0 = (inv2pi / (1.0 + np.abs(wv))).astype(np.float32)
                c1s = (bv * inv2pi).astype(np.float32)
                c1c = (bv * inv2pi + 0.25).astype(np.float32)
                col = 4 * h + 2 * qk
                cstv[0:64, col] = c0
                cstv[64:128, col] = c0
                cstv[0:64, col + 1] = c1c
                cstv[64:128, col + 1] = c1s

        ohv = np.zeros((97, 256), dtype=np.float32)
        for hp in range(2):
            ohv[32 * (2 * hp), hp * 128:hp * 128 + 64] = 1.0
            ohv[32 * (2 * hp + 1), hp * 128 + 64:hp * 128 + 128] = 1.0

        in_maps.append(dict(
            xb=xb_bf, xdup=xdupv, vw=vwt, ow=owp, cst=cstv, tri=tri, oh=ohv,
        ))
    return in_maps


def kernel(x, w_q, b_q, w_k, b_k, v_w, out_w, _trace=False):
    x = np.asarray(x, dtype=np.float32)
    w_q = np.asarray(w_q, dtype=np.float32)
    b_q = np.asarray(b_q, dtype=np.float32)
    w_k = np.asarray(w_k, dtype=np.float32)
    b_k = np.asarray(b_k, dtype=np.float32)
    v_w = np.asarray(v_w, dtype=np.float32)
    out_w = np.asarray(out_w, dtype=np.float32)

    if "nc" not in _CACHE:
        _CACHE["nc"] = _build_nc()
    nc = _CACHE["nc"]

    in_maps = _prep_inputs(x, w_q, b_q, w_k, b_k, v_w, out_w)
    res = run_bass_kernel_spmd(
        nc, in_maps, core_ids=list(range(NCORES)), trace=_trace
    )
    out = np.zeros((B, S, D), dtype=np.float32)
    for c in range(NCORES):
        out[c // 4] += res.results[c]["outT"].T
    if _trace:
        kernel._last_result = res
    return out
